# revision 16
# baseline (speedup 1.0000x reference)
"""Trainium2 Bass kernel for the BindingGNN (EGNN) problem.

Self-contained: kernel(**inputs) -> [G, 1] float32.

8-way SPMD on one trn2 chip:
  - Nodes padded to NP = 8*NPC; core k owns nodes [NPC*k, NPC*(k+1)).
  - Edges live on the core owning their target (row), grouped by 128-node
    block, padded to a uniform W slots per block.
  - Node state (h[128] bf16 + pos[3] bf16) lives in a packed DRAM table of
    512B rows per core; per-edge sources fetched with gpsimd.dma_gather
    (transposed output feeds matmuls directly).  The int16 index limit is
    handled by a lo/hi table split with dedicated zero rows, so the two
    half-gathers simply sum.
  - Row-side expansion and segment sums are one-hot matmuls (Sexp/Sagg).
  - Biases ride ScalarE activations; deg normalization is folded into the
    per-edge gate scalar.
  - Per layer, updated (h,pos) slices are AllGathered and re-expanded into
    each core's table.
  - Precision: bf16 + fp32 PSUM; selected weights are split bf16 hi+lo.
"""
import numpy as np


# ---------------------------------------------------------------- config ---
class Cfg:
    def __init__(self, n_nodes, n_edges, n_graphs, nblk_per_core,
                 w_override=None):
        self.NCORES = 8
        self.H = 128
        self.BLK = 128
        self.TILE = 512
        self.NBLK = nblk_per_core
        self.NPC = nblk_per_core * self.BLK
        self.NP = self.NCORES * self.NPC
        self.SPLIT = self.NP // 2
        self.TBL_ROWS = self.NP + 2
        self.TBL_ELEM = 256
        self.N = n_nodes
        self.E = n_edges
        self.G = n_graphs
        self.L = 4
        self.EPS = 1e-5
        self.w_override = w_override
        assert self.NP >= n_nodes


# ---------------------------------------------------------- preprocessing ---
def _wrap_idx(idx, num):
    a = np.asarray(idx, np.int16).reshape(num // 16, 16).T
    return np.tile(a, (8, 1)).copy()


def preprocess(cfg, x, pos, edge_index, edge_attr, batch, is_protein,
               Wp, bp, Wl, bl, bn_gamma, bn_beta,
               e_w1, e_b1, e_w2, e_b2, c_w1, c_b1, c_w2,
               n_w1, n_b1, n_w2, n_b2,
               h1_w, h1_b, h2_w, h2_b, h3_w, h3_b):
    c = cfg
    N, E, G = c.N, c.E, c.G
    H, BLK, TILE, NBLK, NPC, NP = c.H, c.BLK, c.TILE, c.NBLK, c.NPC, c.NP
    NCORES, SPLIT = c.NCORES, c.SPLIT
    EA = edge_attr.shape[1]
    LN = Wl.shape[0]
    row = edge_index[0].astype(np.int64)
    col = edge_index[1].astype(np.int64)

    P = {'cfg': c}
    core_of_edge = row // NPC
    blk_of_edge = (row % NPC) // BLK
    cnt = np.zeros((NCORES, NBLK), np.int64)
    np.add.at(cnt, (core_of_edge, blk_of_edge), 1)
    W = ((int(cnt.max()) + TILE - 1) // TILE) * TILE
    if c.w_override:
        W = c.w_override
    P['W'] = W
    E_pad = NBLK * W
    P['E_pad'] = E_pad
    P['NT'] = E_pad // TILE
    P['TPB'] = W // TILE
    NCH = E_pad // BLK
    P['NCH'] = NCH

    order = np.lexsort((np.arange(E), blk_of_edge, core_of_edge))
    deg = np.maximum(np.bincount(row, minlength=N).astype(np.float32), 1.0)
    ZLO = SPLIT
    ZHI = SPLIT

    def tbl_row_of_node(n):
        n = np.asarray(n)
        return np.where(n < SPLIT, n, n + 1)
    P['tbl_row_of_node'] = tbl_row_of_node

    per_core = []
    eptr = 0
    for k in range(NCORES):
        ncore = int((core_of_edge == k).sum())
        eidx = order[eptr:eptr + ncore]
        eptr += ncore
        r_k, c_k, ea_k = row[eidx], col[eidx], edge_attr[eidx]
        valid = np.zeros(E_pad, bool)
        rloc = np.zeros(E_pad, np.int64)
        cglob = np.zeros(E_pad, np.int64)
        ea_pad = np.zeros((E_pad, EA), np.float32)
        for b in range(NBLK):
            sel = (r_k % NPC) // BLK == b
            nb = int(sel.sum())
            s = b * W
            valid[s:s + nb] = True
            rloc[s:s + nb] = r_k[sel] % NPC
            cglob[s:s + nb] = c_k[sel]
            ea_pad[s:s + nb] = ea_k[sel]
        is_lo = cglob < SPLIT
        idx_lo = np.where(valid & is_lo, cglob, ZLO)
        idx_hi = np.where(valid & ~is_lo, cglob - SPLIT, ZHI)
        rin = np.where(valid, rloc % BLK, -1)

        Sexp = np.zeros((NBLK, BLK, W), np.float32)
        m = rin >= 0
        bb = np.arange(E_pad) // W
        ee = np.arange(E_pad) % W
        Sexp[bb[m], rin[m], ee[m]] = 1.0
        Sagg = np.zeros((NCH, BLK, BLK), np.float32)
        cc2 = np.arange(E_pad) // BLK
        ee2 = np.arange(E_pad) % BLK
        Sagg[cc2[m], ee2[m], rin[m]] = 1.0
        Sagg_t = Sagg.reshape(P['NT'], TILE // BLK, BLK, BLK) \
                     .transpose(0, 2, 1, 3).copy()

        node_base = k * NPC
        deg_own = np.ones(NPC, np.float32)
        nreal = max(0, min(NPC, N - node_base))
        deg_own[:nreal] = deg[node_base:node_base + nreal]
        dinv_edge = np.where(valid, 1.0 / deg_own[rloc], 0.0).astype(np.float32)
        dinv_w = dinv_edge.reshape(NCH, BLK).T.copy()

        pc = dict(idx_lo=idx_lo, idx_hi=idx_hi, eaT=ea_pad.T.copy(),
                  Sexp=Sexp, Sagg_t=Sagg_t, dinv=dinv_w,
                  valid=valid, rloc=rloc, cglob=cglob)
        for key, idx in (('wlo', idx_lo), ('whi', idx_hi)):
            t = idx.reshape(P['NT'], TILE)
            wt = np.zeros((P['NT'], 128, TILE // 16), np.int16)
            for i in range(P['NT']):
                wt[i] = _wrap_idx(t[i], TILE)
            pc[key] = wt.transpose(1, 0, 2).reshape(128, -1).copy()
        per_core.append(pc)
    P['per_core'] = per_core

    ip = is_protein.astype(np.float32)[:, None]
    xaug = np.concatenate([x * ip, ip, x[:, :LN] * (1 - ip), 1 - ip], 1)
    KIN = xaug.shape[1]
    P['KIN'] = KIN
    xaug_pad = np.zeros((NP, KIN), np.float32)
    xaug_pad[:N] = xaug
    P['xaugT'] = [xaug_pad[k * NPC:(k + 1) * NPC].T.copy() for k in range(NCORES)]
    P['W_cat'] = np.concatenate([Wp, bp[None], Wl, bl[None]], 0).astype(np.float32)

    A = e_w1[:, :H, :]
    B = e_w1[:, H:2 * H, :]
    w_d2 = e_w1[:, 2 * H, :]
    w_ea = e_w1[:, 2 * H + 1:, :]
    P['A'], P['B'] = A, B
    P['rest'] = np.concatenate([np.repeat(w_d2[:, None, :], 3, 1), w_ea], 1)
    P['REST_K'] = 3 + EA
    P['e_b1'], P['e_w2'], P['e_b2'] = e_b1, e_w2, e_b2
    P['c_w1'], P['c_b1'], P['c_w2'] = c_w1, c_b1, c_w2
    P['n_w1a'], P['n_w1b'] = n_w1[:, :H, :], n_w1[:, H:, :]
    P['n_b1'], P['n_w2'], P['n_b2'] = n_b1, n_w2, n_b2
    P['bn_gamma'], P['bn_beta'] = bn_gamma, bn_beta
    P['inv_n'] = float(1.0 / N)

    tbl = np.zeros((c.TBL_ROWS, c.TBL_ELEM), np.float32)
    pos_pad = np.zeros((NP, 3), np.float32)
    pos_pad[:N] = pos
    tbl[tbl_row_of_node(np.arange(NP)), H:H + 3] = pos_pad
    P['table0'] = tbl
    P['pos0'] = [pos_pad[k * NPC:(k + 1) * NPC].reshape(NBLK, BLK, 3)
                 .transpose(1, 0, 2).copy() for k in range(NCORES)]

    gids_list = []
    ngr = 0
    for k in range(NCORES):
        nb = k * NPC
        nreal = max(0, min(NPC, N - nb))
        gl = batch[nb:nb + nreal].astype(np.int64) if nreal else np.zeros(0, np.int64)
        gids = np.unique(gl)
        gids_list.append(gids)
        ngr = max(ngr, len(gids))
    NGR = max(4, ((ngr + 3) // 4) * 4)
    P['NGR'] = NGR
    assert NCORES * NGR <= 128
    Sg_all = []
    for k in range(NCORES):
        nb = k * NPC
        gids = gids_list[k]
        lut = {g: i for i, g in enumerate(gids)}
        g_of = np.full(NPC, -1, np.int64)
        nreal = max(0, min(NPC, N - nb))
        if nreal:
            g_of[:nreal] = [lut[g] for g in batch[nb:nb + nreal].astype(np.int64)]
        Sg = np.zeros((NBLK, BLK, NGR), np.float32)
        for b in range(NBLK):
            gg = g_of[b * BLK:(b + 1) * BLK]
            mm = gg >= 0
            Sg[b, np.arange(BLK)[mm], gg[mm]] = 1.0
        Sg_all.append(Sg)
    P['Sg'] = Sg_all
    cntg = np.maximum(np.bincount(batch.astype(np.int64), minlength=G), 1) \
             .astype(np.float32)
    A_cat = np.zeros((128, 2 * G), np.float32)
    for k in range(NCORES):
        for i, g in enumerate(gids_list[k]):
            A_cat[k * NGR + i, g] = 1.0
            A_cat[k * NGR + i, G + g] = 1.0 / cntg[g]
    P['A_cat'] = A_cat
    P['h1_w'], P['h1_b'] = h1_w, h1_b
    P['h2_w'], P['h2_b'] = h2_w, h2_b
    P['h3_w'], P['h3_b'] = h3_w, h3_b
    return P


# ------------------------------------------------------------- bass build ---
def build(P):
    import sys
    for pth in ('/opt/trn_rl_repo',):
        if pth not in sys.path:
            sys.path.insert(0, pth)
    from concourse import mybir
    from concourse.bacc import Bacc
    from concourse.tile import TileContext

    c = P['cfg']
    H, BLK, TILE, NBLK, NPC = c.H, c.BLK, c.TILE, c.NBLK, c.NPC
    L, NCORES, G, NGR = c.L, c.NCORES, c.G, P['NGR']
    W, NT, TPB, NCH = P['W'], P['NT'], P['TPB'], P['NCH']
    E_pad, KIN, RK = P['E_pad'], P['KIN'], P['REST_K']
    CPT = TILE // BLK
    BF = mybir.dt.bfloat16
    F32 = mybir.dt.float32
    I16 = mybir.dt.int16
    SILU = mybir.ActivationFunctionType.Silu
    RELU = mybir.ActivationFunctionType.Relu
    IDEN = mybir.ActivationFunctionType.Identity
    SQUARE = mybir.ActivationFunctionType.Square
    SQRT = mybir.ActivationFunctionType.Sqrt
    XYZW = mybir.AxisListType.XYZW
    ADD = mybir.AluOpType.add

    nc = Bacc(None, target_bir_lowering=False, num_swdge_queues=2)
    dp = nc.declare_dram_parameter
    table0 = dp("table0", [c.TBL_ROWS, c.TBL_ELEM], BF, isOutput=False)
    xaugT = dp("xaugT", [KIN, NPC], BF, isOutput=False)
    pos0 = dp("pos0", [128, NBLK * 3], F32, isOutput=False)
    dinv = dp("dinv", [128, NCH], F32, isOutput=False)
    wlo = dp("wlo", [128, NT * TILE // 16], I16, isOutput=False)
    whi = dp("whi", [128, NT * TILE // 16], I16, isOutput=False)
    Sexp = dp("Sexp", [NBLK, 128, W], BF, isOutput=False)
    Sagg = dp("Sagg", [NT, 128, CPT, BLK], BF, isOutput=False)
    eaT = dp("eaT", [RK - 3, E_pad], BF, isOutput=False)
    Sg = dp("Sg", [NBLK, 128, NGR], BF, isOutput=False)
    Wcat2 = dp("Wcat2", [KIN, 2, H], BF, isOutput=False)
    Aw = dp("Aw", [128, L, H], BF, isOutput=False)
    Bw = dp("Bw", [128, L, H], BF, isOutput=False)
    restw = dp("restw", [RK, L, H], BF, isOutput=False)
    ew2 = dp("ew2", [H, 2 * L, H], BF, isOutput=False)
    cw1 = dp("cw1", [H, 2 * L, H], BF, isOutput=False)
    cw2 = dp("cw2", [H, L, 2], BF, isOutput=False)
    nw1a = dp("nw1a", [H, 2 * L, H], BF, isOutput=False)
    nw1b = dp("nw1b", [H, 2 * L, H], BF, isOutput=False)
    nw2 = dp("nw2", [H, 2 * L, H], BF, isOutput=False)
    biases = dp("biases", [128, 5 * L], F32, isOutput=False)
    bnw = dp("bnw", [128, 2], F32, isOutput=False)
    A_cat = dp("A_cat", [128, 2, 2 * G], BF, isOutput=False)
    h1w = dp("h1w", [H, 4, 128], BF, isOutput=False)
    h2w = dp("h2w", [128, 2, 64], BF, isOutput=False)
    h3w = dp("h3w", [64, 2, 4], BF, isOutput=False)
    hbias = dp("hbias", [128, 2], F32, isOutput=False)
    h3b = dp("h3b", [4, 1], F32, isOutput=False)
    ident = dp("ident", [128, 128], BF, isOutput=False)
    out_t = dp("out", [4, G], F32, isOutput=True)

    dt = nc.dram_tensor
    table = dt("table", [c.TBL_ROWS, c.TBL_ELEM], BF)
    cc_in = dt("cc_in", [NPC, 132], BF)
    cc_out = dt("cc_out", [NCORES * NPC, 132], BF, addr_space="Shared")
    bn_in = dt("bn_in", [128, 2], F32)
    bn_out = dt("bn_out", [128, 2], F32, addr_space="Shared")
    pool_in = dt("pool_in", [NGR, 128], F32)
    pool_out = dt("pool_out", [NCORES * NGR, 128], F32, addr_space="Shared")

    with TileContext(nc) as tc:
        with tc.tile_pool(name="const", bufs=1) as cpool, \
             tc.tile_pool(name="resid", bufs=1) as rpool, \
             tc.tile_pool(name="stream", bufs=3) as spool, \
             tc.tile_pool(name="work", bufs=2) as wpool, \
             tc.tile_pool(name="ps_z1", bufs=2, space="PSUM") as ps_z1, \
             tc.tile_pool(name="ps_mlp", bufs=2, space="PSUM") as ps_mlp, \
             tc.tile_pool(name="ps_sm", bufs=2, space="PSUM") as ps_sm, \
             tc.tile_pool(name="ps_agg", bufs=1, space="PSUM") as ps_agg, \
             tc.tile_pool(name="ps_pd", bufs=1, space="PSUM") as ps_pd:

            tile_reg = nc.gpsimd.to_reg(TILE)

            def load_const(name, shape, dtype, src):
                t = cpool.tile(shape, dtype, name=name)
                nc.sync.dma_start(out=t[...], in_=src)
                return t

            idlo = load_const("idlo", [128, NT * TILE // 16], I16, wlo[:, :])
            idhi = load_const("idhi", [128, NT * TILE // 16], I16, whi[:, :])
            dinv_sb = load_const("dinv_sb", [128, NCH], F32, dinv[:, :])
            bias_sb = load_const("bias_sb", [128, 5 * L], F32, biases[:, :])
            bn_sb = load_const("bn_sb", [128, 2], F32, bnw[:, :])
            id_sb = load_const("id_sb", [128, 128], BF, ident[:, :])
            wc_sb = load_const("wc_sb", [KIN, 2, H], BF, Wcat2[:, :, :])
            Aw_sb = load_const("Aw_sb", [128, L, H], BF, Aw[:, :, :])
            Bw_sb = load_const("Bw_sb", [128, L, H], BF, Bw[:, :, :])
            rest_sb = load_const("rest_sb", [RK, L, H], BF, restw[:, :, :])
            ew2_sb = load_const("ew2_sb", [H, 2 * L, H], BF, ew2[:, :, :])
            cw1_sb = load_const("cw1_sb", [H, 2 * L, H], BF, cw1[:, :, :])
            cw2_sb = load_const("cw2_sb", [H, L, 2], BF, cw2[:, :, :])
            nw1a_sb = load_const("nw1a_sb", [H, 2 * L, H], BF, nw1a[:, :, :])
            nw1b_sb = load_const("nw1b_sb", [H, 2 * L, H], BF, nw1b[:, :, :])
            nw2_sb = load_const("nw2_sb", [H, 2 * L, H], BF, nw2[:, :, :])

            hT = rpool.tile([128, NPC], BF, name="hT")
            hT2 = rpool.tile([128, NPC], BF, name="hT2")
            agghi = rpool.tile([128, NPC], BF, name="agghi")
            agglo = rpool.tile([128, NPC], BF, name="agglo")
            posf = rpool.tile([128, NBLK, 3], F32, name="posf")
            posb = rpool.tile([128, NBLK, 4], BF, name="posb")
            hrow = rpool.tile([128, NBLK, 128], BF, name="hrow")

            nc.sync.dma_start(out=table[:, :], in_=table0[:, :])
            nc.sync.dma_start(
                out=posf[...],
                in_=pos0[:, :].rearrange("p (b t) -> p b t", b=NBLK))
            nc.vector.memset(posb[...], 0.0)
            nc.vector.tensor_copy(out=posb[:, :, 0:3], in_=posf[...])

            # ---------------- prologue: h0 + BN ----------------
            xa = rpool.tile([KIN, NPC], BF, name="xa")
            nc.sync.dma_start(out=xa[...], in_=xaugT[:, :])
            h0f = rpool.tile([128, NPC], F32, name="h0f")
            NTN = (NPC + TILE - 1) // TILE
            for t in range(NTN):
                s = t * TILE
                e = min(NPC, s + TILE)
                pz = ps_mlp.tile([128, TILE], F32, name="pz0", tag="mlp")
                nc.tensor.matmul(out=pz[:, :e - s], lhsT=wc_sb[:, 0, :],
                                 rhs=xa[:, s:e], start=True, stop=False)
                nc.tensor.matmul(out=pz[:, :e - s], lhsT=wc_sb[:, 1, :],
                                 rhs=xa[:, s:e], start=False, stop=True)
                nc.scalar.copy(out=h0f[:, s:e], in_=pz[:, :e - s])
            ssum = wpool.tile([128, 1], F32, name="ssum")
            nc.vector.reduce_sum(out=ssum[...], in_=h0f[...], axis=mybir.AxisListType.X)
            scratch = wpool.tile([128, NPC], BF, name="scr", tag="bnscr")
            ssq = wpool.tile([128, 1], F32, name="ssq")
            nc.scalar.activation(out=scratch[...], in_=h0f[...], func=SQUARE,
                                 accum_out=ssq[...])
            st2 = wpool.tile([128, 2], F32, name="st2")
            nc.vector.tensor_copy(out=st2[:, 0:1], in_=ssum[...])
            nc.vector.tensor_copy(out=st2[:, 1:2], in_=ssq[...])
            nc.sync.dma_start(out=bn_in[:, :], in_=st2[...])
            nc.gpsimd.collective_compute(
                "AllReduce", ADD,
                replica_groups=[list(range(NCORES))],
                ins=[bn_in.ap().opt()], outs=[bn_out.ap().opt()])
            st2g = wpool.tile([128, 2], F32, name="st2g")
            nc.sync.dma_start(out=st2g[...], in_=bn_out[:, :])
            mu = wpool.tile([128, 1], F32, name="mu")
            nc.scalar.mul(out=mu[...], in_=st2g[:, 0:1], mul=P['inv_n'])
            var = wpool.tile([128, 1], F32, name="var")
            nc.scalar.mul(out=var[...], in_=st2g[:, 1:2], mul=P['inv_n'])
            mu2 = wpool.tile([128, 1], F32, name="mu2")
            nc.vector.tensor_mul(out=mu2[...], in0=mu[...], in1=mu[...])
            nc.vector.tensor_sub(out=var[...], in0=var[...], in1=mu2[...])
            epsb = wpool.tile([128, 1], F32, name="epsb")
            nc.vector.memset(epsb[...], float(c.EPS))
            sd = wpool.tile([128, 1], F32, name="sd")
            nc.scalar.activation(out=sd[...], in_=var[...], func=SQRT,
                                 bias=epsb[...])
            sinv = wpool.tile([128, 1], F32, name="sinv")
            nc.vector.reciprocal(out=sinv[...], in_=sd[...])
            scl = wpool.tile([128, 1], F32, name="scl")
            nc.vector.tensor_mul(out=scl[...], in0=sinv[...], in1=bn_sb[:, 0:1])
            nmu = wpool.tile([128, 1], F32, name="nmu")
            nc.vector.tensor_mul(out=nmu[...], in0=mu[...], in1=scl[...])
            bnb = wpool.tile([128, 1], F32, name="bnb")
            nc.vector.tensor_sub(out=bnb[...], in0=bn_sb[:, 1:2], in1=nmu[...])
            nc.scalar.activation(out=hT[...], in_=h0f[...], func=IDEN,
                                 bias=bnb[...], scale=scl[...])

            def transpose_blocks(src_bf, dst):
                for b in range(NBLK):
                    pt = ps_sm.tile([128, 128], BF, name="pt", tag="sm")
                    nc.tensor.transpose(out=pt[...],
                                        in_=src_bf[:, b * BLK:(b + 1) * BLK],
                                        identity=id_sb[...])
                    nc.scalar.copy(out=dst[:, b, :], in_=pt[...])

            def exchange(h_src):
                transpose_blocks(h_src, hrow)
                nc.sync.dma_start(
                    out=cc_in[:, 0:128].rearrange("(b p) f -> p b f", p=128),
                    in_=hrow[...])
                nc.sync.dma_start(
                    out=cc_in[:, 128:132].rearrange("(b p) f -> p b f", p=128),
                    in_=posb[...])
                nc.gpsimd.collective_compute(
                    "AllGather", mybir.AluOpType.bypass,
                    replica_groups=[list(range(NCORES))],
                    ins=[cc_in.ap().opt()], outs=[cc_out.ap().opt()])
                for k8 in range(NCORES):
                    rb = k8 * NPC + (1 if k8 * NPC >= c.SPLIT else 0)
                    nc.sync.dma_start(
                        out=table[rb:rb + NPC, 0:132],
                        in_=cc_out[k8 * NPC:(k8 + 1) * NPC, :])

            exchange(hT)

            ebias = lambda l, j: bias_sb[:, 5 * l + j:5 * l + j + 1]
            cur, nxt = hT, hT2
            for l in range(L):
                last = (l == L - 1)
                for b in range(NBLK):
                    pu = ps_sm.tile([128, 128], F32, name="pu", tag="sm")
                    nc.tensor.matmul(out=pu[...],
                                     lhsT=cur[:, b * BLK:(b + 1) * BLK],
                                     rhs=Aw_sb[:, l, :], start=True, stop=True)
                    u_sb = wpool.tile([128, 128], BF, name="u_sb", tag="u")
                    nc.scalar.copy(out=u_sb[...], in_=pu[...])
                    aggp = ps_agg.tile([128, BLK], F32, name="aggp", tag="agg")
                    if not last:
                        pdp = ps_pd.tile([3, BLK], F32, name="pdp", tag="pd")
                    for t in range(TPB):
                        ti = b * TPB + t
                        io = ti * (TILE // 16)
                        se = spool.tile([128, TILE], BF, name="se", tag="sexp")
                        nc.sync.dma_start(out=se[...],
                                          in_=Sexp[b, :, t * TILE:(t + 1) * TILE])
                        sa = spool.tile([128, CPT, BLK], BF, name="sa", tag="sagg")
                        nc.sync.dma_start(out=sa[...], in_=Sagg[ti, :, :, :])
                        glo = spool.tile([128, 2, TILE], BF, name="glo", tag="glo")
                        nc.gpsimd.dma_gather(
                            out_ap=glo[...], in_ap=table[:c.SPLIT + 1, :],
                            idxs_ap=idlo[:, io:io + TILE // 16],
                            num_idxs=TILE, num_idxs_reg=tile_reg,
                            elem_size=c.TBL_ELEM, transpose=True, queue_num=0)
                        ghi = spool.tile([128, 2, TILE], BF, name="ghi", tag="ghi")
                        nc.gpsimd.dma_gather(
                            out_ap=ghi[...], in_ap=table[c.SPLIT + 1:, :],
                            idxs_ap=idhi[:, io:io + TILE // 16],
                            num_idxs=TILE, num_idxs_reg=tile_reg,
                            elem_size=c.TBL_ELEM, transpose=True, queue_num=1)
                        pr = ps_sm.tile([3, TILE], F32, name="pr", tag="sm")
                        nc.tensor.matmul(out=pr[...], lhsT=posb[:, b, 0:3],
                                         rhs=se[...], start=True, stop=True)
                        relq = wpool.tile([3, TILE], BF, name="relq", tag="relq")
                        nc.vector.tensor_sub(out=relq[...], in0=pr[...],
                                             in1=glo[0:3, 1, :])
                        nc.vector.tensor_sub(out=relq[...], in0=relq[...],
                                             in1=ghi[0:3, 1, :])
                        rr = wpool.tile([RK, TILE], BF, name="rr", tag="rr")
                        nc.scalar.activation(out=rr[0:3, :], in_=relq[...],
                                             func=SQUARE)
                        nc.sync.dma_start(out=rr[3:RK, :],
                                          in_=eaT[:, ti * TILE:(ti + 1) * TILE])
                        z1 = ps_z1.tile([128, TILE], F32, name="z1", tag="z1")
                        nc.tensor.matmul(out=z1[...], lhsT=u_sb[...], rhs=se[...],
                                         start=True, stop=False)
                        nc.tensor.matmul(out=z1[...], lhsT=Bw_sb[:, l, :],
                                         rhs=glo[:, 0, :], start=False, stop=False)
                        nc.tensor.matmul(out=z1[...], lhsT=Bw_sb[:, l, :],
                                         rhs=ghi[:, 0, :], start=False, stop=False)
                        nc.tensor.matmul(out=z1[...], lhsT=rest_sb[:, l, :],
                                         rhs=rr[...], start=False, stop=True)
                        s1 = wpool.tile([128, TILE], BF, name="s1", tag="s1")
                        nc.scalar.activation(out=s1[...], in_=z1[...], func=SILU,
                                             bias=ebias(l, 0))
                        z2 = ps_mlp.tile([128, TILE], F32, name="z2", tag="mlp")
                        nc.tensor.matmul(out=z2[...], lhsT=ew2_sb[:, 2 * l, :],
                                         rhs=s1[...], start=True, stop=False)
                        nc.tensor.matmul(out=z2[...], lhsT=ew2_sb[:, 2 * l + 1, :],
                                         rhs=s1[...], start=False, stop=True)
                        mT = wpool.tile([128, TILE], BF, name="mTt", tag="mTt")
                        nc.scalar.activation(out=mT[...], in_=z2[...], func=SILU,
                                             bias=ebias(l, 1))
                        if not last:
                            zc = ps_mlp.tile([128, TILE], F32, name="zc", tag="mlp")
                            nc.tensor.matmul(out=zc[...], lhsT=cw1_sb[:, 2 * l, :],
                                             rhs=mT[...], start=True, stop=False)
                            nc.tensor.matmul(out=zc[...], lhsT=cw1_sb[:, 2 * l + 1, :],
                                             rhs=mT[...], start=False, stop=True)
                            c1 = wpool.tile([128, TILE], BF, name="c1", tag="c1")
                            nc.scalar.activation(out=c1[...], in_=zc[...],
                                                 func=SILU, bias=ebias(l, 2))
                        for ch in range(CPT):
                            cs = slice(ch * BLK, (ch + 1) * BLK)
                            chg = b * (W // BLK) + t * CPT + ch
                            pm = ps_sm.tile([128, 128], BF, name="pm", tag="sm")
                            nc.tensor.transpose(out=pm[...], in_=mT[:, cs],
                                                identity=id_sb[...])
                            m_sb = wpool.tile([128, 128], BF, name="m_sb", tag="msb")
                            nc.vector.tensor_copy(out=m_sb[...], in_=pm[...])
                            nc.tensor.matmul(out=aggp[...], lhsT=m_sb[...],
                                             rhs=sa[:, ch, :],
                                             start=(t == 0 and ch == 0),
                                             stop=(t == TPB - 1 and ch == CPT - 1))
                            if last:
                                continue
                            pw = ps_sm.tile([128, 4], F32, name="pw", tag="sm")
                            nc.tensor.matmul(out=pw[:, 0:2], lhsT=c1[:, cs],
                                             rhs=cw2_sb[:, l, :],
                                             start=True, stop=True)
                            pwsb = wpool.tile([128, 2], F32, name="pwsb", tag="pwsb")
                            nc.scalar.copy(out=pwsb[...], in_=pw[:, 0:2])
                            wsc = wpool.tile([128, 1], F32, name="wsc", tag="wsc")
                            nc.vector.tensor_add(out=wsc[...], in0=pwsb[:, 0:1],
                                                 in1=pwsb[:, 1:2])
                            nc.vector.tensor_mul(out=wsc[...], in0=wsc[...],
                                                 in1=dinv_sb[:, chg:chg + 1])
                            prl = ps_sm.tile([128, 3], BF, name="prl", tag="sm")
                            nc.tensor.transpose(out=prl[...], in_=relq[:, cs],
                                                identity=id_sb[0:3, 0:3])
                            rw = wpool.tile([128, 3], BF, name="rw", tag="rw")
                            nc.vector.tensor_scalar_mul(
                                out=rw[...], in0=prl[...], scalar1=wsc[...])
                            nc.tensor.matmul(out=pdp[...], lhsT=rw[...],
                                             rhs=sa[:, ch, :],
                                             start=(t == 0 and ch == 0),
                                             stop=(t == TPB - 1 and ch == CPT - 1))
                    bs = slice(b * BLK, (b + 1) * BLK)
                    nc.vector.tensor_copy(out=agghi[:, bs], in_=aggp[...])
                    alo = wpool.tile([128, BLK], F32, name="alo", tag="alo")
                    nc.vector.tensor_sub(out=alo[...], in0=aggp[...],
                                         in1=agghi[:, bs])
                    nc.vector.tensor_copy(out=agglo[:, bs], in_=alo[...])
                    if not last:
                        pdsb = wpool.tile([3, BLK], BF, name="pdsb", tag="pdsb")
                        nc.vector.tensor_copy(out=pdsb[...], in_=pdp[...])
                        pdt = ps_sm.tile([128, 3], BF, name="pdt", tag="sm")
                        nc.tensor.transpose(out=pdt[...], in_=pdsb[...],
                                            identity=id_sb[0:3, 0:3])
                        nc.vector.tensor_add(out=posf[:, b, :], in0=posf[:, b, :],
                                             in1=pdt[...])
                        nc.vector.tensor_copy(out=posb[:, b, 0:3], in_=posf[:, b, :])
                # node update
                for t in range(NTN):
                    s = t * TILE
                    e = min(NPC, s + TILE)
                    pz = ps_z1.tile([128, TILE], F32, name="pzn", tag="z1")
                    nc.tensor.matmul(out=pz[:, :e - s], lhsT=nw1a_sb[:, 2 * l, :],
                                     rhs=cur[:, s:e], start=True, stop=False)
                    nc.tensor.matmul(out=pz[:, :e - s], lhsT=nw1a_sb[:, 2 * l + 1, :],
                                     rhs=cur[:, s:e], start=False, stop=False)
                    nc.tensor.matmul(out=pz[:, :e - s], lhsT=nw1b_sb[:, 2 * l, :],
                                     rhs=agghi[:, s:e], start=False, stop=False)
                    nc.tensor.matmul(out=pz[:, :e - s], lhsT=nw1b_sb[:, 2 * l + 1, :],
                                     rhs=agghi[:, s:e], start=False, stop=False)
                    nc.tensor.matmul(out=pz[:, :e - s], lhsT=nw1b_sb[:, 2 * l, :],
                                     rhs=agglo[:, s:e], start=False, stop=True)
                    sv = wpool.tile([128, TILE], BF, name="sv", tag="sv")
                    nc.scalar.activation(out=sv[:, :e - s], in_=pz[:, :e - s],
                                         func=SILU, bias=ebias(l, 3))
                    pu2 = ps_mlp.tile([128, TILE], F32, name="pu2", tag="mlp")
                    nc.tensor.matmul(out=pu2[:, :e - s], lhsT=nw2_sb[:, 2 * l, :],
                                     rhs=sv[:, :e - s], start=True, stop=False)
                    nc.tensor.matmul(out=pu2[:, :e - s], lhsT=nw2_sb[:, 2 * l + 1, :],
                                     rhs=sv[:, :e - s], start=False, stop=True)
                    upd = wpool.tile([128, TILE], F32, name="upd", tag="upd")
                    nc.scalar.activation(out=upd[:, :e - s], in_=pu2[:, :e - s],
                                         func=IDEN, bias=ebias(l, 4))
                    nc.vector.tensor_add(out=nxt[:, s:e], in0=cur[:, s:e],
                                         in1=upd[:, :e - s])
                cur, nxt = nxt, cur
                if not last:
                    exchange(cur)

            # ---------------- pooling + head ----------------
            transpose_blocks(cur, hrow)
            ppool = ps_pd.tile([NGR, 128], F32, name="ppool", tag="pd")
            for b in range(NBLK):
                sg_t = spool.tile([128, NGR], BF, name="sg_t", tag="sgt")
                nc.sync.dma_start(out=sg_t[...], in_=Sg[b, :, :])
                nc.tensor.matmul(out=ppool[...], lhsT=sg_t[...],
                                 rhs=hrow[:, b, :], start=(b == 0),
                                 stop=(b == NBLK - 1))
            ppsb = wpool.tile([NGR, 128], F32, name="ppsb", tag="ppsb")
            nc.vector.tensor_copy(out=ppsb[...], in_=ppool[...])
            nc.sync.dma_start(out=pool_in[:, :], in_=ppsb[...])
            nc.gpsimd.collective_compute(
                "AllGather", mybir.AluOpType.bypass,
                replica_groups=[list(range(NCORES))],
                ins=[pool_in.ap().opt()], outs=[pool_out.ap().opt()])
            pall = wpool.tile([128, 128], F32, name="pall", tag="pall")
            if NCORES * NGR < 128:
                nc.vector.memset(pall[...], 0.0)
            nc.sync.dma_start(out=pall[0:NCORES * NGR, :], in_=pool_out[:, :])
            phi = wpool.tile([128, 128], BF, name="phi", tag="phi")
            nc.vector.tensor_copy(out=phi[...], in_=pall[...])
            phif = wpool.tile([128, 128], F32, name="phif", tag="phif")
            nc.vector.tensor_copy(out=phif[...], in_=phi[...])
            plo_f = wpool.tile([128, 128], F32, name="plo_f", tag="plo_f")
            nc.vector.tensor_sub(out=plo_f[...], in0=pall[...], in1=phif[...])
            plo = wpool.tile([128, 128], BF, name="plo", tag="plo")
            nc.vector.tensor_copy(out=plo[...], in_=plo_f[...])
            ac_sb = wpool.tile([128, 2, 2 * G], BF, name="ac_sb", tag="acat")
            nc.sync.dma_start(out=ac_sb[...], in_=A_cat[:, :, :])
            gps = ps_mlp.tile([128, 2 * G], F32, name="gps", tag="mlp")
            nc.tensor.matmul(out=gps[...], lhsT=phi[...], rhs=ac_sb[:, 0, :],
                             start=True, stop=False)
            nc.tensor.matmul(out=gps[...], lhsT=plo[...], rhs=ac_sb[:, 0, :],
                             start=False, stop=False)
            nc.tensor.matmul(out=gps[...], lhsT=phi[...], rhs=ac_sb[:, 1, :],
                             start=False, stop=True)
            gf = wpool.tile([128, 2 * G], BF, name="gf", tag="gf")
            nc.vector.tensor_copy(out=gf[...], in_=gps[...])
            h1_sb = wpool.tile([H, 4, 128], BF, name="h1_sb", tag="h1sb")
            nc.sync.dma_start(out=h1_sb[...], in_=h1w[:, :, :])
            hb_sb = wpool.tile([128, 2], F32, name="hb_sb", tag="hbsb")
            nc.sync.dma_start(out=hb_sb[...], in_=hbias[:, :])
            z1h = ps_mlp.tile([128, G], F32, name="z1h", tag="mlp")
            nc.tensor.matmul(out=z1h[...], lhsT=h1_sb[:, 0, :],
                             rhs=gf[:, 0:G], start=True, stop=False)
            nc.tensor.matmul(out=z1h[...], lhsT=h1_sb[:, 1, :],
                             rhs=gf[:, 0:G], start=False, stop=False)
            nc.tensor.matmul(out=z1h[...], lhsT=h1_sb[:, 2, :],
                             rhs=gf[:, G:2 * G], start=False, stop=False)
            nc.tensor.matmul(out=z1h[...], lhsT=h1_sb[:, 3, :],
                             rhs=gf[:, G:2 * G], start=False, stop=True)
            zr = wpool.tile([128, G], BF, name="zr", tag="zr")
            nc.scalar.activation(out=zr[...], in_=z1h[...], func=RELU,
                                 bias=hb_sb[:, 0:1])
            h2_sb = wpool.tile([128, 2, 64], BF, name="h2_sb", tag="h2sb")
            nc.sync.dma_start(out=h2_sb[...], in_=h2w[:, :, :])
            z2h = ps_mlp.tile([64, G], F32, name="z2h", tag="mlp")
            nc.tensor.matmul(out=z2h[...], lhsT=h2_sb[:, 0, :], rhs=zr[...],
                             start=True, stop=False)
            nc.tensor.matmul(out=z2h[...], lhsT=h2_sb[:, 1, :], rhs=zr[...],
                             start=False, stop=True)
            zr2 = wpool.tile([64, G], BF, name="zr2", tag="zr2")
            nc.scalar.activation(out=zr2[...], in_=z2h[...], func=RELU,
                                 bias=hb_sb[0:64, 1:2])
            h3_sb = wpool.tile([64, 2, 4], BF, name="h3_sb", tag="h3sb")
            nc.sync.dma_start(out=h3_sb[...], in_=h3w[:, :, :])
            h3b_sb = wpool.tile([4, 1], F32, name="h3b_sb", tag="h3bsb")
            nc.sync.dma_start(out=h3b_sb[...], in_=h3b[:, :])
            zo = ps_mlp.tile([4, G], F32, name="zo", tag="mlp")
            nc.tensor.matmul(out=zo[...], lhsT=h3_sb[:, 0, :], rhs=zr2[...],
                             start=True, stop=False)
            nc.tensor.matmul(out=zo[...], lhsT=h3_sb[:, 1, :], rhs=zr2[...],
                             start=False, stop=True)
            zof = wpool.tile([4, G], F32, name="zof", tag="zof")
            nc.scalar.activation(out=zof[...], in_=zo[...], func=IDEN,
                                 bias=h3b_sb[...])
            nc.sync.dma_start(out=out_t[:, :], in_=zof[...])
    nc.compile()
    return nc


# ----------------------------------------------------------- in_maps ---
def make_in_maps(P):
    import ml_dtypes
    BF = ml_dtypes.bfloat16
    c = P['cfg']
    L, H, G = c.L, c.H, c.G

    def split2(w):
        hi = w.astype(BF).astype(np.float32)
        lo = (w - hi).astype(BF)
        return np.stack([hi.astype(BF), lo], 0)

    common = {}
    common['table0'] = P['table0'].astype(BF)
    common['Wcat2'] = split2(P['W_cat']).transpose(1, 0, 2).copy()
    common['Aw'] = P['A'].transpose(1, 0, 2).astype(BF).copy()
    common['Bw'] = P['B'].transpose(1, 0, 2).astype(BF).copy()
    common['restw'] = P['rest'].transpose(1, 0, 2).astype(BF).copy()
    common['ew2'] = np.stack([split2(P['e_w2'][l]) for l in range(L)], 0).transpose(2, 0, 1, 3).reshape(H, 2 * L, H).copy()
    common['cw1'] = np.stack([split2(P['c_w1'][l]) for l in range(L)], 0).transpose(2, 0, 1, 3).reshape(H, 2 * L, H).copy()
    common['cw2'] = np.stack([split2(P['c_w2'][l])[:, :, 0].T for l in range(L)], 0).transpose(1, 0, 2).copy()
    common['nw1a'] = np.stack([split2(P['n_w1a'][l]) for l in range(L)], 0).transpose(2, 0, 1, 3).reshape(H, 2 * L, H).copy()
    common['nw1b'] = np.stack([split2(P['n_w1b'][l]) for l in range(L)], 0).transpose(2, 0, 1, 3).reshape(H, 2 * L, H).copy()
    common['nw2'] = np.stack([split2(P['n_w2'][l]) for l in range(L)], 0).transpose(2, 0, 1, 3).reshape(H, 2 * L, H).copy()
    bias = np.zeros((128, 5 * L), np.float32)
    for l in range(L):
        bias[:, 5 * l + 0] = P['e_b1'][l]
        bias[:, 5 * l + 1] = P['e_b2'][l]
        bias[:, 5 * l + 2] = P['c_b1'][l]
        bias[:, 5 * l + 3] = P['n_b1'][l]
        bias[:, 5 * l + 4] = P['n_b2'][l]
    common['biases'] = bias
    common['bnw'] = np.stack([P['bn_gamma'], P['bn_beta']], 1).astype(np.float32)
    common['A_cat'] = split2(P['A_cat']).transpose(1, 0, 2).copy()
    h1s = split2(P['h1_w'])
    common['h1w'] = np.stack([h1s[:, :H, :], h1s[:, H:, :]], 0).transpose(2, 0, 1, 3).reshape(H, 4, 128).copy()
    common['h2w'] = split2(P['h2_w']).transpose(1, 0, 2).copy()
    h3p = np.zeros((64, 4), np.float32)
    h3p[:, 0:1] = P['h3_w']
    common['h3w'] = split2(h3p).transpose(1, 0, 2).copy()
    common['hbias'] = np.stack([
        P['h1_b'], np.concatenate([P['h2_b'], np.zeros(64, np.float32)])], 1
    ).astype(np.float32)
    h3b = np.zeros((4, 1), np.float32)
    h3b[0, 0] = float(np.asarray(P['h3_b']).ravel()[0])
    common['h3b'] = h3b
    common['ident'] = np.eye(128, dtype=np.float32).astype(BF)

    in_maps = []
    for k in range(c.NCORES):
        pc = P['per_core'][k]
        m = dict(common)
        m['xaugT'] = P['xaugT'][k].astype(BF)
        m['pos0'] = P['pos0'][k].reshape(128, -1).astype(np.float32)
        m['dinv'] = pc['dinv']
        m['wlo'] = pc['wlo']
        m['whi'] = pc['whi']
        m['Sexp'] = pc['Sexp'].astype(BF)
        m['Sagg'] = pc['Sagg_t'].astype(BF)
        m['eaT'] = pc['eaT'].astype(BF)
        m['Sg'] = P['Sg'][k].astype(BF)
        in_maps.append(m)
    return in_maps


# ----------------------------------------------------------- entrypoint ---
def make_cfg(inputs):
    n = inputs['x'].shape[0]
    return Cfg(n, inputs['edge_index'].shape[1],
               int(inputs['batch'].max()) + 1,
               (n + 8 * 128 - 1) // (8 * 128))


def kernel(**inputs):
    import sys
    for pth in ('/opt/trn_rl_repo',):
        if pth not in sys.path:
            sys.path.insert(0, pth)
    inputs = {k: np.asarray(v) for k, v in inputs.items()}
    cfg = make_cfg(inputs)
    P = preprocess(cfg, **inputs)
    nc = build(P)
    in_maps = make_in_maps(P)
    from concourse.bass_utils import run_bass_kernel_spmd
    res = run_bass_kernel_spmd(nc, in_maps, core_ids=list(range(cfg.NCORES)))
    out = np.asarray(res.results[0]['out'])
    return out[0, :cfg.G].reshape(-1, 1).astype(np.float32)
